# revision 10
# baseline (speedup 1.0000x reference)
"""EdgeConv (ParticleNet-style) Trainium2 kernel.

Full inputs: x [128, 512, 32] fp32, W1 [64, 128], b1 [128], W2 [128, 64], b2 [64].
Output: [128, 512, 64] fp32.

Strategy (data-parallel over batch, 16 events per core on 8 cores):
  per event:
    - keys[i, j] = ci.cj - |cj|^2/2  (order-equiv to -dist^2/2) via PE fp32 matmul
      (K=4: [cx, cy, -1, -1] x [cx, cy, cx^2/2, cy^2/2]), diag masked to -BIG
      via a bf16 identity-matmul accumulate of a host-built diag const.
    - top-16 per row: DVE max8 -> match_replace -> max8; indices via max_index.
    - neighbor gather of v' = x @ W1b + b1 (node-major bf16 in SBUF) with
      gpsimd dma_gather(transpose=True) -> feature-major [128h, 8192 edges].
    - h = relu(vgath + p_bcast), p = x @ (W1a - W1b): DVE add + DVE/ACT relu.
    - out.T[64, 512] = sum_k W2'/16 ^T @ relu_slice_k (16 accumulating matmuls)
      + b2 rank-1; host transposes back.
"""

import numpy as np
import ml_dtypes

B, N, F = 128, 512, 32
K = 16
H, OUT = 128, 64
NCORES = 8
EV = B // NCORES  # events per core
BIG = np.float32(1e30)

_cache = {}


def _build_nc(relu_act_cols=2048, n_ev=EV, stage=99):
    import concourse.bass as bass
    import concourse.bacc as bacc
    import concourse.tile as tile
    import concourse.mybir as mybir
    from contextlib import ExitStack

    dt = mybir.dt
    nc = bacc.Bacc("TRN2", target_bir_lowering=False, debug=False,
                   enable_asserts=False, num_devices=NCORES,
                   num_swdge_queues=4)

    # DRAM I/O (per core)
    xt_d = nc.dram_tensor("xt", [n_ev, F, N], dt.float32, kind="ExternalInput")
    wpv_d = nc.dram_tensor("wpv", [F + 1, 2 * H], dt.bfloat16, kind="ExternalInput")
    # wpv: [[Wp | Wv'], [0-row | b1]]  (33 x 256): cols 0:128 = Wp (no bias),
    # cols 128:256 = Wv with b1 in row 32.
    w2b_d = nc.dram_tensor("w2b", [H, OUT], dt.bfloat16, kind="ExternalInput")
    b2s_d = nc.dram_tensor("b2s", [2, OUT], dt.bfloat16, kind="ExternalInput")
    diag_d = nc.dram_tensor("diag", [128, 4, N], dt.bfloat16, kind="ExternalInput")
    ident_d = nc.dram_tensor("ident", [128, 128], dt.bfloat16, kind="ExternalInput")
    out_d = nc.dram_tensor("out", [n_ev, OUT, N], dt.float32, kind="ExternalOutput")

    AF = mybir.ActivationFunctionType

    with tile.TileContext(nc) as tc, ExitStack() as ctx:
        cpool = ctx.enter_context(tc.tile_pool(name="consts", bufs=1))
        ident = cpool.tile([128, 128], dt.bfloat16)
        nc.sync.dma_start(ident[:], ident_d[:])
        diag = cpool.tile([128, 4, N], dt.bfloat16)
        nc.sync.dma_start(diag[:], diag_d[:])
        wpv = cpool.tile([F + 1, 2 * H], dt.bfloat16)
        nc.sync.dma_start(wpv[:], wpv_d[:])
        w2b = cpool.tile([H, OUT], dt.bfloat16)
        nc.sync.dma_start(w2b[:], w2b_d[:])
        b2s = cpool.tile([2, OUT], dt.bfloat16)
        nc.sync.dma_start(b2s[:], b2s_d[:])
        ones2 = cpool.tile([2, N], dt.bfloat16)
        nc.gpsimd.memset(ones2[:], 1.0)
        l4c = cpool.tile([2, N], dt.float32)   # lhsT rows 2-3 = -1 (fp32)
        nc.gpsimd.memset(l4c[:], -1.0)

        xt_pool = ctx.enter_context(tc.tile_pool(name="xt", bufs=2))
        l4_pool = ctx.enter_context(tc.tile_pool(name="l4", bufs=2))
        r4_pool = ctx.enter_context(tc.tile_pool(name="r4", bufs=2))
        sq_pool = ctx.enter_context(tc.tile_pool(name="sq", bufs=2))
        xtb_pool = ctx.enter_context(tc.tile_pool(name="xtb", bufs=2))
        keys_pool = ctx.enter_context(tc.tile_pool(name="keys", bufs=2))
        keys2_pool = ctx.enter_context(tc.tile_pool(name="keys2", bufs=2))
        vals_pool = ctx.enter_context(tc.tile_pool(name="vals", bufs=2))
        idxs_pool = ctx.enter_context(tc.tile_pool(name="idxs", bufs=2))
        idxdg_pool = ctx.enter_context(tc.tile_pool(name="idxdg", bufs=2))
        p_pool = ctx.enter_context(tc.tile_pool(name="p", bufs=2))
        v_pool = ctx.enter_context(tc.tile_pool(name="v", bufs=2))
        vg_pool = ctx.enter_context(tc.tile_pool(name="vg", bufs=2))
        vgr_pool = ctx.enter_context(tc.tile_pool(name="vgr", bufs=2))
        outsb_pool = ctx.enter_context(tc.tile_pool(name="outsb", bufs=2))

        vdram_pool = ctx.enter_context(tc.tile_pool(name="vdram", bufs=2, space="DRAM"))
        kps_pool = ctx.enter_context(tc.tile_pool(name="kps", bufs=2, space="PSUM"))
        pps_pool = ctx.enter_context(tc.tile_pool(name="pps", bufs=2, space="PSUM"))
        hps_pool = ctx.enter_context(tc.tile_pool(name="hps", bufs=1, space="PSUM"))
        ops_pool = ctx.enter_context(tc.tile_pool(name="ops", bufs=1, space="PSUM"))
        hbT_pool = ctx.enter_context(tc.tile_pool(name="hbT", bufs=2))

        for e in range(n_ev):
            # ---- load xT
            xt = xt_pool.tile([F, N], dt.float32)
            nc.sync.dma_start(xt[:], xt_d[e])

            # rhs for key mm: [cx; cy; cx^2/2; cy^2/2]
            r4 = r4_pool.tile([4, N], dt.float32)
            nc.sync.dma_start(r4[0:2, :], xt_d[e][0:2, :])
            sq = sq_pool.tile([2, N], dt.float32)
            nc.scalar.activation(sq[:], xt[0:2, :], AF.Square,
                                 scale=float(np.sqrt(0.5)))
            nc.sync.dma_start(r4[2:4, :], sq[:])
            # lhsT for key mm: [cx; cy; -1; -1] rows on partitions 0-3
            l4 = l4_pool.tile([4, N], dt.float32)
            nc.scalar.activation(l4[0:2, :], r4[0:2, :], AF.Copy)
            nc.sync.dma_start(l4[2:4, :], l4c[:])

            # bf16 xt (+ ones row 32) for p/v matmuls
            xtb = xtb_pool.tile([F + 1, N], dt.bfloat16)
            nc.scalar.activation(xtb[0:F, :], xt[:], AF.Copy)
            nc.gpsimd.memset(xtb[F:F + 1, :], 1.0)

            # ---- p and v' (both node-major [i%128, i//128, h])
            pps = pps_pool.tile([128, N], dt.float32)
            for c in range(4):
                nc.tensor.matmul(pps[:, 128 * c:128 * (c + 1)],
                                 xtb[0:F, 128 * c:128 * (c + 1)],
                                 wpv[0:F, 0:H], start=True, stop=True)
            p_nm = p_pool.tile([128, 4, H], dt.bfloat16)
            nc.scalar.activation(p_nm[:].opt(), pps[:], AF.Copy)

            vps = pps_pool.tile([128, N], dt.float32)
            for c in range(4):
                nc.tensor.matmul(vps[:, 128 * c:128 * (c + 1)],
                                 xtb[0:F + 1, 128 * c:128 * (c + 1)],
                                 wpv[0:F + 1, H:2 * H], start=True, stop=True)
            v_sb = v_pool.tile([128, 4, H], dt.bfloat16)
            nc.scalar.activation(v_sb[:].opt(), vps[:], AF.Copy)
            v_dram = vdram_pool.tile([N, H], dt.bfloat16)
            nc.sync.dma_start(v_dram[:].rearrange("(c q) h -> q c h", c=4, q=128),
                              v_sb[:])

            # ---- keys + selection per 128-row tile
            keys = keys_pool.tile([128, 4, N], dt.float32)
            keys2 = keys2_pool.tile([128, 4, N], dt.float32)
            vals = vals_pool.tile([128, 64], dt.float32)
            idxs = idxs_pool.tile([128, 64], dt.uint16)
            for t in range(4):
                kps = kps_pool.tile([128, N], dt.float32)
                nc.tensor.matmul(kps[:], l4[:, 128 * t:128 * (t + 1)], r4[:],
                                 start=True, stop=False)
                nc.tensor.matmul(kps[:], ident[:], diag[:, t, :],
                                 start=False, stop=True)
                kt = keys[:, t, :].opt()
                k2t = keys2[:, t, :].opt()
                nc.scalar.activation(kt, kps[:], AF.Copy)
                v0 = vals[:, 16 * t:16 * t + 8]
                v1 = vals[:, 16 * t + 8:16 * t + 16]
                # idxs col layout: 4*r + t (r = k-slot 0..15) for regroup DMA
                idxs4 = idxs[:].rearrange("p (r t) -> p r t", r=16, t=4)
                i0 = idxs4[:, 0:8, t].opt()
                i1 = idxs4[:, 8:16, t].opt()
                nc.vector.max(v0, kt)
                nc.vector.match_replace(k2t, v0, kt, -float(BIG))
                nc.vector.max(v1, k2t)
                nc.vector.max_index(i0, v0, kt)
                nc.vector.max_index(i1, v1, k2t)

            if stage <= 2:
                nc.sync.dma_start(out_d[e][:, 0:64].rearrange("a b -> a b"), 
                                  vals[0:OUT, :].bitcast(dt.float32)) if False else None
                dbg = keys[:, 0, :].opt()
                nc.sync.dma_start(out_d[e][0:OUT, :], dbg[0:OUT, :])
                continue

            # ---- regroup idx -> dma_gather layout [16, 512] then replicate
            # logical edge e2 = k*512 + i  ->  partition e2%16, slot e2//16
            # idx_dg[p, r*32 + 8t + u] = idxs[16u+p, 4r + t]
            idx_dg = idxdg_pool.tile([128, N], dt.uint16)
            for u in range(8):
                src_ap = idxs[16 * u:16 * (u + 1), :]          # [16, 64] (r,t)
                dst_ap = idx_dg[0:16, :].rearrange(
                    "p (a b) -> p a b", a=64, b=8)[:, :, u].opt()  # [16,64] step 8
                nc.sync.dma_start(dst_ap, src_ap)
            # log2 replication: 16 -> 32 -> 64 -> 128 partitions
            nc.sync.dma_start(idx_dg[16:32, :], idx_dg[0:16, :])
            nc.sync.dma_start(idx_dg[32:64, :], idx_dg[0:32, :])
            nc.sync.dma_start(idx_dg[64:128, :], idx_dg[0:64, :])

            if stage <= 3:
                nc.sync.dma_start(out_d[e][0:16, :],
                                  idx_dg[0:16, :].bitcast(dt.float32)) if False else None
                dbg16 = idx_dg[0:64, :].bitcast(dt.float32)
                nc.sync.dma_start(out_d[e][0:32, :], dbg16[0:64, 0:256].rearrange("a b -> a b")) if False else None
                nc.sync.dma_start(out_d[e][:, :], keys2[:, 0:2, :].opt()[0:OUT, 0:N])
                continue

            # ---- gather v' by idx (SBUF source, transposed out: feature-major)
            # node-major gather: vg[p, r, :] = v'[idx[e]], e = r*128 + p = k*512 + i
            vg = vg_pool.tile([128, K * N // 128, H], dt.bfloat16)
            for s8 in range(8):
                nc.gpsimd.dma_gather(
                    out_ap=vg[:, s8 * 8:(s8 + 1) * 8, :],
                    in_ap=v_dram[:],
                    idxs_ap=idx_dg[:, s8 * 64:(s8 + 1) * 64].bitcast(dt.int16),
                    num_idxs=1024,
                    num_idxs_reg=1024,
                    elem_size=H,
                    transpose=False,
                    single_packet=True,
                    queue_num=s8 % 4,
                )

            if stage <= 4:
                nc.sync.dma_start(out_d[e][:, :],
                                  vg[:].opt()[0:OUT, 0:2 * N].bitcast(dt.float32))
                continue

            # ---- edges: h = relu(vg + p_i) node-major; rank r = k*4 + c
            vg4 = vg[:].rearrange("p (k c) h -> p k c h", k=K, c=4)
            p_b = p_nm[:].rearrange("p (k c) h -> p k c h", k=1, c=4).broadcast_to(
                [128, K, 4, H])
            nc.vector.tensor_tensor(vg4, vg4, p_b, op=mybir.AluOpType.add)
            vgr = vgr_pool.tile([128, K * N // 128, H], dt.bfloat16)
            vgf = vg[:].opt()   # [128, 8192]
            vgrf = vgr[:].opt()
            ac = relu_act_cols
            if ac > 0:
                nc.scalar.activation(vgrf[:, 0:ac], vgf[:, 0:ac], AF.Relu)
            nc.vector.tensor_scalar_max(vgrf[:, ac:K * H * 4], vgf[:, ac:K * H * 4],
                                        0.0)

            # ---- hbar.T via PE transpose-accumulate: psum[h, i-chunk c]
            hps = hps_pool.tile([128, 4, 128], dt.float32)
            for c in range(4):
                for k in range(K):
                    nc.tensor.matmul(hps[:, c, :].opt(),
                                     vgr[:, k * 4 + c, :].opt(),
                                     ident[:], start=(k == 0), stop=(k == K - 1))
            hbT = hbT_pool.tile([128, 4 * 128], dt.bfloat16)
            nc.scalar.activation(hbT[:], hps[:].opt(), AF.Copy)

            # ---- layer 2: out.T[64, N] = W2'.T @ hbar.T + b2
            ops = ops_pool.tile([OUT, N], dt.float32)
            nc.tensor.matmul(ops[:], w2b[:], hbT[:], start=True, stop=False)
            nc.tensor.matmul(ops[:], b2s[:], ones2[:],
                             start=False, stop=True)
            osb = outsb_pool.tile([OUT, N], dt.float32)
            nc.scalar.activation(osb[:], ops[:], AF.Copy)
            nc.sync.dma_start(out_d[e], osb[:])

    nc.compile()
    return nc


def _prep_inputs(x, W1, b1, W2, b2):
    bf16 = ml_dtypes.bfloat16
    Wp = (W1[0:F, :] - W1[F:2 * F, :]).astype(np.float32)
    Wv = W1[F:2 * F, :].astype(np.float32)
    wpv = np.zeros((F + 1, 2 * H), dtype=bf16)
    wpv[0:F, 0:H] = Wp.astype(bf16)
    wpv[0:F, H:2 * H] = Wv.astype(bf16)
    wpv[F, H:2 * H] = b1.astype(bf16)

    w2b = (W2.astype(np.float32) / np.float32(K)).astype(bf16)
    b2f = b2.astype(np.float32)
    b2hi = b2f.astype(bf16)
    b2lo = (b2f - b2hi.astype(np.float32)).astype(bf16)
    b2s = np.stack([b2hi, b2lo]).astype(bf16)

    diag = np.zeros((128, 4, N), dtype=bf16)
    for t in range(4):
        diag[np.arange(128), t, t * 128 + np.arange(128)] = bf16(-BIG)

    ident = np.eye(128, dtype=bf16)

    xt = np.ascontiguousarray(x.transpose(0, 2, 1).astype(np.float32))  # [B, F, N]
    return xt, wpv, w2b, b2s, diag, ident


def kernel(x, W1, b1, W2, b2):
    from concourse.bass_utils import run_bass_kernel_spmd

    key = "nc"
    if key not in _cache:
        _cache[key] = _build_nc()
    nc = _cache[key]

    xt, wpv, w2b, b2s, diag, ident = _prep_inputs(
        np.asarray(x), np.asarray(W1), np.asarray(b1),
        np.asarray(W2), np.asarray(b2))

    in_maps = []
    for c in range(NCORES):
        in_maps.append({
            "xt": xt[c * EV:(c + 1) * EV],
            "wpv": wpv, "w2b": w2b, "b2s": b2s, "diag": diag, "ident": ident,
        })
    res = run_bass_kernel_spmd(nc, in_maps, list(range(NCORES)))
    outs = [res.results[c]["out"] for c in range(NCORES)]  # [EV, OUT, N]
    full = np.concatenate(outs, axis=0)                    # [B, OUT, N]
    return np.ascontiguousarray(full.transpose(0, 2, 1)).astype(np.float32)



# revision 11
# speedup vs baseline: 1.1409x; 1.1409x over previous
"""EdgeConv (ParticleNet-style) Trainium2 kernel.

Full inputs: x [128, 512, 32] fp32, W1 [64, 128], b1 [128], W2 [128, 64], b2 [64].
Output: [128, 512, 64] fp32.

Strategy (data-parallel over batch, 16 events per core on 8 cores):
  per event:
    - keys[i, j] = ci.cj - |cj|^2/2  (order-equiv to -dist^2/2) via PE fp32 matmul
      (K=4: [cx, cy, -1, -1] x [cx, cy, cx^2/2, cy^2/2]), diag masked to -BIG
      via a bf16 identity-matmul accumulate of a host-built diag const.
    - top-16 per row: DVE max8 -> match_replace -> max8; indices via max_index.
    - neighbor gather of v' = x @ W1b + b1 (node-major bf16 in SBUF) with
      gpsimd dma_gather(transpose=True) -> feature-major [128h, 8192 edges].
    - h = relu(vgath + p_bcast), p = x @ (W1a - W1b): DVE add + DVE/ACT relu.
    - out.T[64, 512] = sum_k W2'/16 ^T @ relu_slice_k (16 accumulating matmuls)
      + b2 rank-1; host transposes back.
"""

import numpy as np
import ml_dtypes

B, N, F = 128, 512, 32
K = 16
H, OUT = 128, 64
NCORES = 8
EV = B // NCORES  # events per core
BIG = np.float32(1e30)

_cache = {}


def _build_nc(relu_act_cols=2048, n_ev=EV, stage=99):
    import concourse.bass as bass
    import concourse.bacc as bacc
    import concourse.tile as tile
    import concourse.mybir as mybir
    from contextlib import ExitStack

    dt = mybir.dt
    nc = bacc.Bacc("TRN2", target_bir_lowering=False, debug=False,
                   enable_asserts=False, num_devices=NCORES,
                   num_swdge_queues=4)

    # DRAM I/O (per core)
    xt_d = nc.dram_tensor("xt", [n_ev, F, N], dt.float32, kind="ExternalInput")
    wpv_d = nc.dram_tensor("wpv", [F + 1, 2 * H], dt.bfloat16, kind="ExternalInput")
    # wpv: [[Wp | Wv'], [0-row | b1]]  (33 x 256): cols 0:128 = Wp (no bias),
    # cols 128:256 = Wv with b1 in row 32.
    w2b_d = nc.dram_tensor("w2b", [H, OUT], dt.bfloat16, kind="ExternalInput")
    b2s_d = nc.dram_tensor("b2s", [2, OUT], dt.bfloat16, kind="ExternalInput")
    diag_d = nc.dram_tensor("diag", [128, 4, N], dt.bfloat16, kind="ExternalInput")
    ident_d = nc.dram_tensor("ident", [128, 128], dt.bfloat16, kind="ExternalInput")
    out_d = nc.dram_tensor("out", [n_ev, OUT, N], dt.float32, kind="ExternalOutput")

    AF = mybir.ActivationFunctionType

    with tile.TileContext(nc) as tc, ExitStack() as ctx:
        cpool = ctx.enter_context(tc.tile_pool(name="consts", bufs=1))
        ident = cpool.tile([128, 128], dt.bfloat16)
        nc.sync.dma_start(ident[:], ident_d[:])
        diag = cpool.tile([128, 4, N], dt.bfloat16)
        nc.sync.dma_start(diag[:], diag_d[:])
        wpv = cpool.tile([F + 1, 2 * H], dt.bfloat16)
        nc.sync.dma_start(wpv[:], wpv_d[:])
        w2b = cpool.tile([H, OUT], dt.bfloat16)
        nc.sync.dma_start(w2b[:], w2b_d[:])
        b2s = cpool.tile([2, OUT], dt.bfloat16)
        nc.sync.dma_start(b2s[:], b2s_d[:])
        ones2 = cpool.tile([2, N], dt.bfloat16)
        nc.gpsimd.memset(ones2[:], 1.0)
        l4c = cpool.tile([2, N], dt.float32)   # lhsT rows 2-3 = -1 (fp32)
        nc.gpsimd.memset(l4c[:], -1.0)

        xt_pool = ctx.enter_context(tc.tile_pool(name="xt", bufs=2))
        l4_pool = ctx.enter_context(tc.tile_pool(name="l4", bufs=2))
        r4_pool = ctx.enter_context(tc.tile_pool(name="r4", bufs=2))
        sq_pool = ctx.enter_context(tc.tile_pool(name="sq", bufs=2))
        xtb_pool = ctx.enter_context(tc.tile_pool(name="xtb", bufs=2))
        keys_pool = ctx.enter_context(tc.tile_pool(name="keys", bufs=2))
        keys2_pool = ctx.enter_context(tc.tile_pool(name="keys2", bufs=2))
        vals_pool = ctx.enter_context(tc.tile_pool(name="vals", bufs=2))
        idxs_pool = ctx.enter_context(tc.tile_pool(name="idxs", bufs=2))
        idxdg_pool = ctx.enter_context(tc.tile_pool(name="idxdg", bufs=2))
        p_pool = ctx.enter_context(tc.tile_pool(name="p", bufs=2))
        v_pool = ctx.enter_context(tc.tile_pool(name="v", bufs=2))
        vg_pool = ctx.enter_context(tc.tile_pool(name="vg", bufs=2))
        vgr_pool = ctx.enter_context(tc.tile_pool(name="vgr", bufs=2))
        outsb_pool = ctx.enter_context(tc.tile_pool(name="outsb", bufs=2))

        vdram_pool = ctx.enter_context(tc.tile_pool(name="vdram", bufs=2, space="DRAM"))
        kps_pool = ctx.enter_context(tc.tile_pool(name="kps", bufs=2, space="PSUM"))
        pps_pool = ctx.enter_context(tc.tile_pool(name="pps", bufs=2, space="PSUM"))
        hps_pool = ctx.enter_context(tc.tile_pool(name="hps", bufs=1, space="PSUM"))
        ops_pool = ctx.enter_context(tc.tile_pool(name="ops", bufs=1, space="PSUM"))
        hbT_pool = ctx.enter_context(tc.tile_pool(name="hbT", bufs=2))

        for e in range(n_ev):
            # ---- load xT
            xt = xt_pool.tile([F, N], dt.float32)
            nc.sync.dma_start(xt[:], xt_d[e])

            # rhs for key mm: [cx; cy; cx^2/2; cy^2/2]
            r4 = r4_pool.tile([4, N], dt.float32)
            nc.sync.dma_start(r4[0:2, :], xt_d[e][0:2, :])
            sq = sq_pool.tile([2, N], dt.float32)
            nc.scalar.activation(sq[:], xt[0:2, :], AF.Square,
                                 scale=float(np.sqrt(0.5)))
            nc.sync.dma_start(r4[2:4, :], sq[:])
            # lhsT for key mm: [cx; cy; -1; -1] rows on partitions 0-3
            l4 = l4_pool.tile([4, N], dt.float32)
            nc.sync.dma_start(l4[0:2, :], xt_d[e][0:2, :])
            nc.sync.dma_start(l4[2:4, :], l4c[:])

            # bf16 xt (+ ones row 32) for p/v matmuls
            xtb = xtb_pool.tile([F + 1, N], dt.bfloat16)
            nc.scalar.activation(xtb[0:F, :], xt[:], AF.Copy)
            nc.gpsimd.memset(xtb[F:F + 1, :], 1.0)

            # ---- p and v' (both node-major [i%128, i//128, h])
            pps = pps_pool.tile([128, N], dt.float32)
            for c in range(4):
                nc.tensor.matmul(pps[:, 128 * c:128 * (c + 1)],
                                 xtb[0:F, 128 * c:128 * (c + 1)],
                                 wpv[0:F, 0:H], start=True, stop=True)
            p_nm = p_pool.tile([128, 4, H], dt.bfloat16)
            nc.scalar.activation(p_nm[:].opt(), pps[:], AF.Copy)

            vps = pps_pool.tile([128, N], dt.float32)
            for c in range(4):
                nc.tensor.matmul(vps[:, 128 * c:128 * (c + 1)],
                                 xtb[0:F + 1, 128 * c:128 * (c + 1)],
                                 wpv[0:F + 1, H:2 * H], start=True, stop=True)
            v_sb = v_pool.tile([128, 4, H], dt.bfloat16)
            nc.scalar.activation(v_sb[:].opt(), vps[:], AF.Copy)
            v_dram = vdram_pool.tile([N, H], dt.bfloat16)
            nc.sync.dma_start(v_dram[:].rearrange("(c q) h -> q c h", c=4, q=128),
                              v_sb[:])

            # ---- keys + selection per 128-row tile
            keys = keys_pool.tile([128, 4, N], dt.float32)
            keys2 = keys2_pool.tile([128, 4, N], dt.float32)
            vals = vals_pool.tile([128, 64], dt.float32)
            idxs = idxs_pool.tile([128, 64], dt.uint16)
            for t in range(4):
                kps = kps_pool.tile([128, N], dt.float32)
                nc.tensor.matmul(kps[:], l4[:, 128 * t:128 * (t + 1)], r4[:],
                                 start=True, stop=False)
                nc.tensor.matmul(kps[:], ident[:], diag[:, t, :],
                                 start=False, stop=True)
                kt = keys[:, t, :].opt()
                k2t = keys2[:, t, :].opt()
                nc.scalar.activation(kt, kps[:], AF.Copy)
                v0 = vals[:, 16 * t:16 * t + 8]
                v1 = vals[:, 16 * t + 8:16 * t + 16]
                # idxs col layout: 4*r + t (r = k-slot 0..15) for regroup DMA
                idxs4 = idxs[:].rearrange("p (r t) -> p r t", r=16, t=4)
                i0 = idxs4[:, 0:8, t].opt()
                i1 = idxs4[:, 8:16, t].opt()
                nc.vector.max(v0, kt)
                nc.vector.match_replace(k2t, v0, kt, -float(BIG))
                nc.vector.max(v1, k2t)
                nc.vector.max_index(i0, v0, kt)
                nc.vector.max_index(i1, v1, k2t)

            if stage <= 2:
                nc.sync.dma_start(out_d[e][:, 0:64].rearrange("a b -> a b"), 
                                  vals[0:OUT, :].bitcast(dt.float32)) if False else None
                dbg = keys[:, 0, :].opt()
                nc.sync.dma_start(out_d[e][0:OUT, :], dbg[0:OUT, :])
                continue

            # ---- regroup idx -> dma_gather layout [16, 512] then replicate
            # logical edge e2 = k*512 + i  ->  partition e2%16, slot e2//16
            # idx_dg[p, r*32 + 8t + u] = idxs[16u+p, 4r + t]
            idx_dg = idxdg_pool.tile([128, N], dt.uint16)
            for u in range(8):
                src_ap = idxs[16 * u:16 * (u + 1), :]          # [16, 64] (r,t)
                dst_ap = idx_dg[0:16, :].rearrange(
                    "p (a b) -> p a b", a=64, b=8)[:, :, u].opt()  # [16,64] step 8
                nc.sync.dma_start(dst_ap, src_ap)
            # replicate [0:16] to the other 7 groups (independent DMAs)
            for g in range(1, 8):
                nc.sync.dma_start(idx_dg[16 * g:16 * (g + 1), :], idx_dg[0:16, :])

            if stage <= 3:
                nc.sync.dma_start(out_d[e][0:16, :],
                                  idx_dg[0:16, :].bitcast(dt.float32)) if False else None
                dbg16 = idx_dg[0:64, :].bitcast(dt.float32)
                nc.sync.dma_start(out_d[e][0:32, :], dbg16[0:64, 0:256].rearrange("a b -> a b")) if False else None
                nc.sync.dma_start(out_d[e][:, :], keys2[:, 0:2, :].opt()[0:OUT, 0:N])
                continue

            # ---- gather v' by idx (SBUF source, transposed out: feature-major)
            # node-major gather: vg[p, r, :] = v'[idx[e]], e = r*128 + p = k*512 + i
            vg = vg_pool.tile([128, K * N // 128, H], dt.bfloat16)
            for s8 in range(8):
                nc.gpsimd.dma_gather(
                    out_ap=vg[:, s8 * 8:(s8 + 1) * 8, :],
                    in_ap=v_dram[:],
                    idxs_ap=idx_dg[:, s8 * 64:(s8 + 1) * 64].bitcast(dt.int16),
                    num_idxs=1024,
                    num_idxs_reg=1024,
                    elem_size=H,
                    transpose=False,
                    single_packet=True,
                    queue_num=s8 % 4,
                )

            if stage <= 4:
                nc.sync.dma_start(out_d[e][:, :],
                                  vg[:].opt()[0:OUT, 0:2 * N].bitcast(dt.float32))
                continue

            # ---- edges: h = relu(vg + p_i) node-major; rank r = k*4 + c
            vg4 = vg[:].rearrange("p (k c) h -> p k c h", k=K, c=4)
            p_b = p_nm[:].rearrange("p (k c) h -> p k c h", k=1, c=4).broadcast_to(
                [128, K, 4, H])
            nc.vector.tensor_tensor(vg4, vg4, p_b, op=mybir.AluOpType.add)
            vgr = vgr_pool.tile([128, K * N // 128, H], dt.bfloat16)
            vgf = vg[:].opt()   # [128, 8192]
            vgrf = vgr[:].opt()
            ac = relu_act_cols
            if ac > 0:
                nc.scalar.activation(vgrf[:, 0:ac], vgf[:, 0:ac], AF.Relu)
            nc.vector.tensor_scalar_max(vgrf[:, ac:K * H * 4], vgf[:, ac:K * H * 4],
                                        0.0)

            # ---- hbar.T via PE transpose-accumulate: psum[h, i-chunk c]
            hps = hps_pool.tile([128, 4, 128], dt.float32)
            for c in range(4):
                for k in range(K):
                    nc.tensor.matmul(hps[:, c, :].opt(),
                                     vgr[:, k * 4 + c, :].opt(),
                                     ident[:], start=(k == 0), stop=(k == K - 1))
            hbT = hbT_pool.tile([128, 4 * 128], dt.bfloat16)
            nc.scalar.activation(hbT[:], hps[:].opt(), AF.Copy)

            # ---- layer 2: out.T[64, N] = W2'.T @ hbar.T + b2
            ops = ops_pool.tile([OUT, N], dt.float32)
            nc.tensor.matmul(ops[:], w2b[:], hbT[:], start=True, stop=False)
            nc.tensor.matmul(ops[:], b2s[:], ones2[:],
                             start=False, stop=True)
            osb = outsb_pool.tile([OUT, N], dt.float32)
            nc.scalar.activation(osb[:], ops[:], AF.Copy)
            nc.sync.dma_start(out_d[e], osb[:])

    nc.compile()
    return nc


def _prep_inputs(x, W1, b1, W2, b2):
    bf16 = ml_dtypes.bfloat16
    Wp = (W1[0:F, :] - W1[F:2 * F, :]).astype(np.float32)
    Wv = W1[F:2 * F, :].astype(np.float32)
    wpv = np.zeros((F + 1, 2 * H), dtype=bf16)
    wpv[0:F, 0:H] = Wp.astype(bf16)
    wpv[0:F, H:2 * H] = Wv.astype(bf16)
    wpv[F, H:2 * H] = b1.astype(bf16)

    w2b = (W2.astype(np.float32) / np.float32(K)).astype(bf16)
    b2f = b2.astype(np.float32)
    b2hi = b2f.astype(bf16)
    b2lo = (b2f - b2hi.astype(np.float32)).astype(bf16)
    b2s = np.stack([b2hi, b2lo]).astype(bf16)

    diag = np.zeros((128, 4, N), dtype=bf16)
    for t in range(4):
        diag[np.arange(128), t, t * 128 + np.arange(128)] = bf16(-BIG)

    ident = np.eye(128, dtype=bf16)

    xt = np.ascontiguousarray(x.transpose(0, 2, 1).astype(np.float32))  # [B, F, N]
    return xt, wpv, w2b, b2s, diag, ident


def kernel(x, W1, b1, W2, b2):
    from concourse.bass_utils import run_bass_kernel_spmd

    key = "nc"
    if key not in _cache:
        _cache[key] = _build_nc()
    nc = _cache[key]

    xt, wpv, w2b, b2s, diag, ident = _prep_inputs(
        np.asarray(x), np.asarray(W1), np.asarray(b1),
        np.asarray(W2), np.asarray(b2))

    in_maps = []
    for c in range(NCORES):
        in_maps.append({
            "xt": xt[c * EV:(c + 1) * EV],
            "wpv": wpv, "w2b": w2b, "b2s": b2s, "diag": diag, "ident": ident,
        })
    res = run_bass_kernel_spmd(nc, in_maps, list(range(NCORES)))
    outs = [res.results[c]["out"] for c in range(NCORES)]  # [EV, OUT, N]
    full = np.concatenate(outs, axis=0)                    # [B, OUT, N]
    return np.ascontiguousarray(full.transpose(0, 2, 1)).astype(np.float32)

